# revision 3
# baseline (speedup 1.0000x reference)
"""EnhancedContrastiveLoss on 8 Trainium2 NeuronCores (Bass/Tile), v2.

Differences vs the v1 baseline (187.7 us):
  * fp16 operands end-to-end on the matmul path (host ships et as fp16;
    norms/normalize on device): PE runs at 1 col/cycle, DVE elementwise
    ops hit the 2x (TT) / 4x (TS) perf modes, input DMA halves.
  * normalization without the DRAM round-trip: ones-matmul replicates
    n2 across partitions in PSUM, ACT sqrt -> sqrtb, TT-divide (2x)
    normalizes in one pass.
  * self-sim is killed pre-exp (-1e30 into PSUM diag): exp gives an
    exact 0 in both the stored E and the ACT row-sum accumulator, so
    the e_self cancellation term disappears entirely.
  * E is bf16: the top-8-negatives path folds the row pairwise with
    TT-max at 2x (8192 -> 1024) before the (modeless, 1x) InstMax.
  * band positives via tensor_tensor_reduce (mult/add -> pos_sum) and a
    reduce-max; same-class mask ships from the host (pure layout).
  * exp in 2048-wide PSUM regions (2 regions ping-pong across the 8
    PSUM banks) with fused row-sum accumulation.

Per-core engine budget (cost model): PE ~57us, ACT ~72us, DVE ~62us.
"""

import numpy as np
from contextlib import ExitStack

import concourse.bass as bass
import concourse.mybir as mybir
from concourse import bacc, tile
from concourse.bass_utils import run_bass_kernel_spmd

F32 = mybir.dt.float32
F16 = mybir.dt.float16
BF16 = mybir.dt.bfloat16
AF = mybir.ActivationFunctionType
ALU = mybir.AluOpType
AX = mybir.AxisListType

B = 8192
D = 256
NC = 8
RPC = B // NC          # rows per core (1024)
NT = RPC // 128        # row tiles per core (8)
KT = D // 128          # K tiles (2)
W = 2048               # PSUM region width (4 banks)
NW = B // W            # regions per row tile (4)
BAND = 384
TEMP = 0.07
MARGIN = 0.2
INVT = 1.0 / TEMP
NEG_BIG = -1.0e30

_CACHE = {}

_TABLES_PATCHED = False


def _patch_act_tables():
    global _TABLES_PATCHED
    if _TABLES_PATCHED:
        return
    from concourse import bacc as _bacc_mod
    _orig = _bacc_mod.get_activation_tables

    def _patched(arch):
        t = _orig(arch)
        for name, fns in t.items():
            if name != "natural_log_exp_and_others":
                fns.discard(AF.Exp)
                fns.discard(AF.Ln)
        return t

    _bacc_mod.get_activation_tables = _patched
    _TABLES_PATCHED = True


def _build_program():
    if "nc" in _CACHE:
        return _CACHE["nc"]
    _patch_act_tables()
    nc = bacc.Bacc(
        "TRN2", target_bir_lowering=False, debug=False, num_devices=NC
    )
    et_d = nc.dram_tensor("et", [D, B], F16, kind="ExternalInput").ap()
    mask_d = nc.dram_tensor("mask01", [128, NT * BAND], F16,
                            kind="ExternalInput").ap()
    eye_d = nc.dram_tensor("eyeneg", [128, 128], F16, kind="ExternalInput").ap()
    pad_d = nc.dram_tensor("eyepad", [128, 1024], F16,
                           kind="ExternalInput").ap()
    out_d = nc.dram_tensor("out", [128, 32], F32, kind="ExternalOutput").ap()

    with tile.TileContext(nc) as tc:
        with ExitStack() as ctx:
            _body(ctx, tc, et_d, mask_d, eye_d, pad_d, out_d)

    nc.finalize()
    _CACHE["nc"] = nc
    return nc


def _body(ctx, tc, et_d, mask_d, eye_d, pad_d, out_d):
    nc = tc.nc

    singles = ctx.enter_context(tc.tile_pool(name="singles", bufs=1))
    etpool = ctx.enter_context(tc.tile_pool(name="etp", bufs=1))
    sqpool = ctx.enter_context(tc.tile_pool(name="sq", bufs=2))
    epool = ctx.enter_context(tc.tile_pool(name="ep", bufs=2))
    bandpool = ctx.enter_context(tc.tile_pool(name="band", bufs=2))
    psmm = ctx.enter_context(tc.tile_pool(name="psmm", bufs=2, space="PSUM"))

    # ---- persistent tiles ----
    ones128 = singles.tile([128, 128], F16)
    etn = [singles.tile([128, B], F16, name=f"etn{k}") for k in range(KT)]
    invb = singles.tile([128, B], F16)
    mask01 = singles.tile([128, NT * BAND], F16)
    eyeneg = singles.tile([128, 128], F16)
    eyepad = singles.tile([128, 1024], F16)
    asum = singles.tile([128, NT * NW], F32)
    psumS = singles.tile([128, NT], F32)
    pmES = singles.tile([128, NT], F32)
    top8s = singles.tile([128, NT * 8], BF16)
    outsb = singles.tile([128, 32], F32)

    nc.gpsimd.memset(ones128[:], 1.0)
    nc.sync.dma_start(mask01[:], mask_d[:, :])
    nc.sync.dma_start(eyeneg[:], eye_d[:, :])
    nc.sync.dma_start(eyepad[:], pad_d[:, :])

    # ---- input DMAs (region-major so the prologue can stream) ----
    et_sb = [etpool.tile([128, B], F16, name=f"et{k}") for k in range(KT)]
    for r in range(NW):
        for k in range(KT):
            nc.sync.dma_start(
                et_sb[k][:, r * W:(r + 1) * W],
                et_d[k * 128:(k + 1) * 128, r * W:(r + 1) * W],
            )

    # ---- prologue: normalize ----
    # inv = 1/sqrt(n2) = exp(-0.5*ln(n2)), both on ACT. InstReciprocal on
    # DVE is microcoded at ~6.3 ns/elem (13 us per region -- unusable) and
    # Pool rejects TensorTensor, so the transcendental path wins. All Ln
    # before all Exp so the activation table loads only twice.
    lnpool = ctx.enter_context(tc.tile_pool(name="lnp", bufs=NW))
    lnb = []
    for r in range(NW):
        c0 = r * W
        sq = [sqpool.tile([128, W], F16, tag="sq", name=f"sq{k}")
              for k in range(KT)]
        for k in range(KT):
            nc.vector.tensor_tensor(
                out=sq[k][:],
                in0=et_sb[k][:, c0:c0 + W],
                in1=et_sb[k][:, c0:c0 + W],
                op=ALU.mult,
            )
        ps = psmm.tile([128, W], F32, tag="mm", name=f"n2ps{r}")
        for h in range(W // 512):
            for k in range(KT):
                nc.tensor.matmul(
                    ps[:, h * 512:(h + 1) * 512],
                    ones128[:],
                    sq[k][:, h * 512:(h + 1) * 512],
                    start=(k == 0), stop=(k == KT - 1),
                )
        ln = lnpool.tile([128, W], F32, tag="ln", name=f"ln{r}")
        nc.scalar.activation(ln[:], ps[:], AF.Ln)
        lnb.append(ln)
    for r in range(NW):
        c0 = r * W
        nc.scalar.activation(invb[:, c0:c0 + W], lnb[r][:], AF.Exp,
                             scale=-0.5)
        for k in range(KT):
            nc.vector.tensor_tensor(
                out=etn[k][:, c0:c0 + W],
                in0=et_sb[k][:, c0:c0 + W],
                in1=invb[:, c0:c0 + W],
                op=ALU.mult,
            )

    # ---- main loop over row tiles ----
    for t in range(NT):
        E = epool.tile([128, B], BF16, tag="E")
        lo = 128 + t * 128          # stationary: this tile's rows as columns
        dg = 128 + t * 128          # global col of this tile's diagonal
        for w in range(NW):
            ps = psmm.tile([128, W], F32, tag="mm")
            for h in range(W // 512):
                c0 = w * W + h * 512
                diag_here = w == 0 and h == dg // 512
                for k in range(KT):
                    nc.tensor.matmul(
                        ps[:, h * 512:(h + 1) * 512],
                        etn[k][:, lo:lo + 128],
                        etn[k][:, c0:c0 + 512],
                        start=(k == 0),
                        stop=(k == KT - 1) and not diag_here,
                    )
                if diag_here:
                    # kill self-sim before exp (exact 0 in E and accum)
                    # with a third accumulation term on the PE itself:
                    # (-2500*I) @ shifted-identity lands -2500 on the diag,
                    # keeping DVE off the PE->ACT critical path.
                    dgl = dg % 512
                    nc.tensor.matmul(
                        ps[:, h * 512:(h + 1) * 512],
                        eyeneg[:],
                        eyepad[:, 512 - dgl:1024 - dgl],
                        start=False, stop=True,
                    )
            nc.scalar.activation(
                E[:, w * W:(w + 1) * W], ps[:], AF.Exp,
                scale=INVT,
                accum_out=asum[:, t * NW + w:t * NW + w + 1],
            )

        # band [t*128, t*128+384): all same-class cols of these rows
        bl = t * 128
        Eb = E[:, bl:bl + BAND]
        m01 = mask01[:, t * BAND:(t + 1) * BAND]
        epos = bandpool.tile([128, BAND], BF16, tag="epos")
        # pos_sum (mask includes self; E_ii == 0 so it's harmless)
        nc.vector.scalar_tensor_tensor(
            out=epos[:], in0=m01[:], scalar=1.0, in1=Eb,
            op0=ALU.mult, op1=ALU.mult,
            accum_out=psumS[:, t:t + 1],
        )
        nc.vector.tensor_reduce(
            out=pmES[:, t:t + 1], in_=epos[:], axis=AX.X, op=ALU.max,
        )
        # mask same-class (incl self) out of E for the negatives top-k
        nc.vector.scalar_tensor_tensor(
            out=Eb, in0=m01[:], scalar=NEG_BIG, in1=Eb,
            op0=ALU.mult, op1=ALU.add,
        )

        # pairwise max-fold 8192 -> 1024 in place, then hardware top-8
        nc.vector.tensor_tensor(
            out=E[:, 0:4096], in0=E[:, 0:4096], in1=E[:, 4096:8192],
            op=ALU.max)
        nc.vector.tensor_tensor(
            out=E[:, 0:2048], in0=E[:, 0:2048], in1=E[:, 2048:4096],
            op=ALU.max)
        nc.vector.tensor_tensor(
            out=E[:, 0:1024], in0=E[:, 0:1024], in1=E[:, 1024:2048],
            op=ALU.max)
        nc.vector.tensor_tensor(
            out=E[:, 0:512], in0=E[:, 0:512], in1=E[:, 512:1024],
            op=ALU.max)
        nc.vector.max(top8s[:, t * 8:(t + 1) * 8], E[:, 0:512])

    # ---- epilogue: per-row losses on [128, NT] tiles ----
    ep = ctx.enter_context(tc.tile_pool(name="epi", bufs=1))
    allsum = ep.tile([128, NT], F32)
    rp = ep.tile([128, NT], F32)
    ratio = ep.tile([128, NT], F32)
    Lb = ep.tile([128, NT], F32)
    hp = ep.tile([128, NT], F32)
    pmx = ep.tile([128, NT], F32)
    l3 = ep.tile([128, NT * 3], F32)
    s123 = ep.tile([128, NT], F32)
    u = ep.tile([128, NT], F32)
    v = ep.tile([128, NT], F32)

    nc.vector.tensor_reduce(
        out=allsum[:], in_=asum[:].rearrange("p (t n) -> p t n", n=NW),
        axis=AX.X, op=ALU.add,
    )
    nc.vector.tensor_scalar_add(allsum[:], allsum[:], 1e-10)
    nc.vector.reciprocal(rp[:], allsum[:])
    nc.vector.tensor_tensor(
        out=ratio[:], in0=psumS[:], in1=rp[:], op=ALU.mult,
    )
    nc.vector.tensor_scalar_add(ratio[:], ratio[:], 1e-10)
    nc.scalar.activation(Lb[:], ratio[:], AF.Ln)
    # hp = has_pos: E_ii == 0 exactly, so pos_sum > 0 iff a real positive
    nc.vector.tensor_scalar(
        out=hp[:], in0=psumS[:], scalar1=0.0, scalar2=None,
        op0=ALU.is_gt,
    )
    # pos_max (ln units); rows with no positives get a junk finite value
    nc.vector.tensor_scalar_max(pmES[:], pmES[:], 1e-30)
    nc.scalar.activation(pmx[:], pmES[:], AF.Ln)
    # top-3 negative sims (ln units)
    l8 = ep.tile([128, NT * 8], F32)
    nc.scalar.activation(l8[:], top8s[:], AF.Ln)
    nc.vector.tensor_reduce(
        out=s123[:],
        in_=l8[:].rearrange("p (t k) -> p t k", k=8)[:, :, 0:3],
        axis=AX.X, op=ALU.add,
    )
    # hard: h = relu(s123/3 - pmx + MARGIN) * hp
    nc.vector.scalar_tensor_tensor(
        out=u[:], in0=s123[:], scalar=1.0 / 3.0, in1=pmx[:],
        op0=ALU.mult, op1=ALU.subtract,
    )
    nc.vector.tensor_scalar(
        out=v[:], in0=u[:], scalar1=MARGIN, scalar2=0.0,
        op0=ALU.add, op1=ALU.max,
    )
    nc.vector.tensor_tensor(
        out=outsb[:, 16:24], in0=v[:], in1=hp[:], op=ALU.mult,
    )
    # margin: m = relu(s1 - pmx + MARGIN) * hp
    nc.vector.scalar_tensor_tensor(
        out=u[:], in0=l8[:].rearrange("p (t k) -> p t k", k=8)[:, :, 0],
        scalar=1.0, in1=pmx[:], op0=ALU.mult, op1=ALU.subtract,
    )
    nc.vector.tensor_scalar(
        out=v[:], in0=u[:], scalar1=MARGIN, scalar2=0.0,
        op0=ALU.add, op1=ALU.max,
    )
    nc.vector.tensor_tensor(
        out=outsb[:, 24:32], in0=v[:], in1=hp[:], op=ALU.mult,
    )
    # basic: -ln(ratio) * hp
    nc.vector.scalar_tensor_tensor(
        out=outsb[:, 0:8], in0=Lb[:], scalar=-1.0, in1=hp[:],
        op0=ALU.mult, op1=ALU.mult,
    )
    nc.vector.tensor_copy(out=outsb[:, 8:16], in_=hp[:])

    nc.sync.dma_start(out_d[:, :], outsb[:])


def _prep_inputs(embeddings, labels):
    e = np.ascontiguousarray(np.asarray(embeddings), dtype=np.float32)
    lab = np.asarray(labels)
    assert e.shape == (B, D) and lab.shape == (B,)
    perm = np.argsort(lab, kind="stable")
    e_s = e[perm]
    lab_s = lab[perm].astype(np.int64)
    counts = np.bincount(lab_s)
    assert counts.max() <= 128, f"class size {counts.max()} > band margin"

    # -2500 (not -1e30): after the INVT scale, exp underflows to an exact
    # 0 in fp32; the -1e30 variant hits a HW failure the softer value avoids
    eyeneg = np.zeros((128, 128), dtype=np.float16)
    eyeneg[np.arange(128), np.arange(128)] = -2500.0
    # identity at cols [512, 640): slicing [512-dgl : 1024-dgl] places it
    # at local col dgl of a 512-wide matmul moving operand
    eyepad = np.zeros((128, 1024), dtype=np.float16)
    eyepad[np.arange(128), 512 + np.arange(128)] = 1.0

    in_maps = []
    for c in range(NC):
        s = (c * RPC - 128) % B
        er = np.concatenate([e_s[s:], e_s[:s]], axis=0)
        lr = np.concatenate([lab_s[s:], lab_s[:s]])
        mask01 = np.zeros((128, NT * BAND), dtype=np.float32)
        for t in range(NT):
            rows = lr[128 + t * 128:256 + t * 128]
            cols = lr[t * 128:t * 128 + BAND]
            mask01[:, t * BAND:(t + 1) * BAND] = (
                rows[:, None] == cols[None, :]
            )
        in_maps.append(
            {
                "et": np.ascontiguousarray(er.T).astype(np.float16),
                "mask01": mask01.astype(np.float16),
                "eyeneg": eyeneg,
                "eyepad": eyepad,
            }
        )
    return in_maps


def _combine(results):
    SA = np.float32(0.0)
    SB = np.float32(0.0)
    SC = np.float32(0.0)
    SD = np.float32(0.0)
    for r in results:
        o = r["out"].astype(np.float32)
        SA += o[:, 0:8].sum(dtype=np.float32)
        SB += o[:, 8:16].sum(dtype=np.float32)
        SC += o[:, 16:24].sum(dtype=np.float32)
        SD += o[:, 24:32].sum(dtype=np.float32)
    nhp = max(SB, np.float32(1.0))
    basic = SA / nhp
    hard = SC / nhp
    margin = SD / nhp if SB > 0 else np.float32(0.0)
    total = basic + np.float32(0.5) * hard + np.float32(0.1) * margin
    return np.asarray(total, dtype=np.float32)


def kernel(embeddings, labels):
    in_maps = _prep_inputs(embeddings, labels)
    nc = _build_program()
    res = run_bass_kernel_spmd(nc, in_maps, core_ids=list(range(NC)))
    return _combine(res.results)
